# revision 5
# baseline (speedup 1.0000x reference)
"""VQ codebook (nn_CodeBook) Trainium2 Bass kernel.

Data-parallel over the N=32768 input rows across 8 NeuronCores; the
[8192, 512] codebook is replicated.  Each core computes, for its 4096-row
shard:

  u    = x @ (2W)^T                    (PE, fp32, 4 d-chunks x 16 k-chunks)
  z    = u - ||w||^2                   (DVE, fused with PSUM evacuation)
  m    = rowmax(z)                     (DVE, per-chunk max + combine)
  e    = exp(z - m), s = rowsum(e)     (ACT, fused accumulate)
  probs= e * (1/s)                     (ACT copy with per-partition scale)
  idx  = argmax(z) == argmin(dist)     (DVE, is_ge mask * reverse iota)
  q    = W[idx]                        (indirect DMA gather)
  st   = x + (q - x)                   (straight-through estimator)
  sse  = rowsum((q - x)^2)             (ACT square with accumulate)

Host side: shard/transpose inputs, then assemble probs/quantized_st,
bincount(idx) -> perplexity, and sum(sse) -> loss.  Softmax of
-(||x||^2 + ||w||^2 - 2xw) over k equals softmax of (2xw - ||w||^2)
because the per-row ||x||^2 shifts cancel; the argmin is likewise
unaffected.
"""

import numpy as np

import concourse.bass as bass
import concourse.bacc as bacc
import concourse.tile as tile
import concourse.mybir as mybir
from concourse.bass_utils import run_bass_kernel_spmd

# Problem shape (hardcoded per contract).
N_INPUTS = 32768
K = 8192            # NUM_EMBEDDINGS
D = 512             # EMBEDDING_DIM
COMMITMENT_COST = 0.25

N_CORES = 8
N_CORE = N_INPUTS // N_CORES   # 4096 rows per core
P = 128                        # partitions
NT = N_CORE // P               # 32 row-tiles per core
KC = 512                       # k chunk (one PSUM bank of fp32)
NKC = K // KC                  # 16 chunks
ND = D // P                    # 4 contraction chunks

f32 = mybir.dt.float32
i32 = mybir.dt.int32


def build_nc():
    nc = bacc.Bacc("TRN2", num_devices=N_CORES, dynamic_dma_scratch_size=8192)

    x_d = nc.dram_tensor("x", [N_CORE, D], f32, kind="ExternalInput")
    xt_d = nc.dram_tensor("xt", [D, N_CORE], f32, kind="ExternalInput")
    wt2_d = nc.dram_tensor("wt2", [D, K], f32, kind="ExternalInput")
    w_d = nc.dram_tensor("w", [K, D], f32, kind="ExternalInput")
    wsq_d = nc.dram_tensor("wsq", [1, K], f32, kind="ExternalInput")

    probs_o = nc.dram_tensor("probs", [N_CORE, K], f32, kind="ExternalOutput")
    st_o = nc.dram_tensor("st", [N_CORE, D], f32, kind="ExternalOutput")
    idx_o = nc.dram_tensor("idx", [N_CORE, 1], i32, kind="ExternalOutput")
    sse_o = nc.dram_tensor("sse", [P, NT], f32, kind="ExternalOutput")

    with tile.TileContext(nc) as tc:
        with (
            tc.tile_pool(name="const", bufs=1) as const,
            tc.tile_pool(name="zc", bufs=16) as zpool,
            tc.tile_pool(name="xin", bufs=2) as xin,
            tc.tile_pool(name="work", bufs=2) as work,
            tc.tile_pool(name="small", bufs=2) as small,
            tc.tile_pool(name="psum", bufs=4, space="PSUM") as psum,
            tc.tile_pool(name="ptmp", bufs=2, space="PSUM") as ptmp,
        ):
            # ---------------- resident constants ----------------
            wt2 = []
            for d in range(ND):
                t = const.tile([P, K], f32, tag=f"wt2_{d}")
                nc.sync.dma_start(out=t[:], in_=wt2_d.ap()[d * P:(d + 1) * P, :])
                wt2.append(t)
            wsqb = const.tile([P, K], f32)
            nc.sync.dma_start(
                out=wsqb[:],
                in_=bass.AP(tensor=wsq_d.ap().tensor, offset=0, ap=[[0, P], [1, K]]),
            )
            iota_rev = const.tile([P, KC], i32)   # KC..1
            nc.gpsimd.iota(iota_rev[:], [[-1, KC]], base=KC, channel_multiplier=0)
            iota_rev_c = const.tile([P, NKC], i32)  # NKC..1
            nc.gpsimd.iota(iota_rev_c[:], [[-1, NKC]], base=NKC, channel_multiplier=0)

            for rt in range(NT):
                rs = slice(rt * P, (rt + 1) * P)
                # ---------------- loads ----------------
                xt = []
                for d in range(ND):
                    t = xin.tile([P, P], f32, tag=f"xt_{d}")
                    nc.sync.dma_start(
                        out=t[:],
                        in_=xt_d.ap()[d * P:(d + 1) * P, rs])
                    xt.append(t)
                x = xin.tile([P, D], f32, tag="x")
                nc.sync.dma_start(out=x[:], in_=x_d.ap()[rs, :])

                # ---------------- matmul + evac + chunk stats ----------------
                zs = []
                M = small.tile([P, NKC], f32, tag="M")
                for c in range(NKC):
                    cs = slice(c * KC, (c + 1) * KC)
                    pu = psum.tile([P, KC], f32, tag="pu")
                    for d in range(ND):
                        nc.tensor.matmul(
                            pu[:], lhsT=xt[d][:], rhs=wt2[d][:, cs],
                            start=(d == 0), stop=(d == ND - 1))
                    z = zpool.tile([P, KC], f32, tag="z")
                    zs.append(z)
                    # z_c = pu - wsq_c  (PSUM -> SBUF)
                    nc.vector.scalar_tensor_tensor(
                        out=z[:], in0=pu[:], scalar=1.0, in1=wsqb[:, cs],
                        op0=mybir.AluOpType.mult, op1=mybir.AluOpType.subtract)
                    nc.vector.tensor_reduce(
                        out=M[:, c:c + 1], in_=z[:], axis=mybir.AxisListType.X,
                        op=mybir.AluOpType.max)

                # ---------------- row max ----------------
                m = small.tile([P, 1], f32, tag="m")
                nc.vector.tensor_reduce(
                    out=m[:], in_=M[:], axis=mybir.AxisListType.X,
                    op=mybir.AluOpType.max)
                negm = small.tile([P, 1], f32, tag="negm")
                nc.vector.tensor_scalar_mul(negm[:], m[:], -1.0)

                # ---------------- argmax masks / exp / normalize ----------------
                J = small.tile([P, NKC], f32, tag="J")
                S = small.tile([P, NKC], f32, tag="S")
                for c in range(NKC):
                    z = zs[c]
                    tmp = ptmp.tile([P, KC], f32, tag="tmp", space="PSUM")
                    nc.vector.scalar_tensor_tensor(
                        out=tmp[:], in0=z[:], scalar=m[:], in1=iota_rev[:],
                        op0=mybir.AluOpType.is_ge, op1=mybir.AluOpType.mult)
                    nc.vector.tensor_reduce(
                        out=J[:, c:c + 1], in_=tmp[:], axis=mybir.AxisListType.X,
                        op=mybir.AluOpType.max)
                    # e_c = exp(z_c - m), accumulate chunk sum
                    nc.scalar.activation(
                        out=z[:], in_=z[:], func=mybir.ActivationFunctionType.Exp,
                        bias=negm[:], scale=1.0, accum_out=S[:, c:c + 1])
                s = small.tile([P, 1], f32, tag="s")
                nc.vector.tensor_reduce(
                    out=s[:], in_=S[:], axis=mybir.AxisListType.X,
                    op=mybir.AluOpType.add)
                r = small.tile([P, 1], f32, tag="r")
                nc.vector.reciprocal(r[:], s[:])
                for c in range(NKC):
                    cs = slice(c * KC, (c + 1) * KC)
                    z = zs[c]
                    nc.scalar.activation(
                        out=z[:], in_=z[:], func=mybir.ActivationFunctionType.Copy,
                        bias=0.0, scale=r[:])
                    nc.sync.dma_start(out=probs_o.ap()[rs, cs], in_=z[:])

                # ---------------- index assembly ----------------
                jrevm = small.tile([P, 1], f32, tag="jrevm")
                nc.vector.tensor_reduce(
                    out=jrevm[:], in_=J[:], axis=mybir.AxisListType.X,
                    op=mybir.AluOpType.max)
                crev = small.tile([P, NKC], f32, tag="crev")
                nc.vector.scalar_tensor_tensor(
                    out=crev[:], in0=J[:], scalar=0.0, in1=iota_rev_c[:],
                    op0=mybir.AluOpType.is_gt, op1=mybir.AluOpType.mult)
                crevm = small.tile([P, 1], f32, tag="crevm")
                nc.vector.tensor_reduce(
                    out=crevm[:], in_=crev[:], axis=mybir.AxisListType.X,
                    op=mybir.AluOpType.max)
                # idx = (NKC*KC + KC) - KC*crevm - jrevm
                idxf = small.tile([P, 1], f32, tag="idxf")
                nc.vector.scalar_tensor_tensor(
                    out=idxf[:], in0=crevm[:], scalar=-float(KC), in1=jrevm[:],
                    op0=mybir.AluOpType.mult, op1=mybir.AluOpType.subtract)
                nc.vector.tensor_scalar_add(idxf[:], idxf[:], float(NKC * KC + KC))
                idxi = small.tile([P, 1], i32, tag="idxi")
                nc.vector.tensor_copy(out=idxi[:], in_=idxf[:])
                nc.sync.dma_start(out=idx_o.ap()[rs, :], in_=idxi[:])

                # ---------------- gather / straight-through / sse ----------------
                q = work.tile([P, D], f32, tag="q")
                nc.gpsimd.indirect_dma_start(
                    out=q[:], out_offset=None, in_=w_d.ap(),
                    in_offset=bass.IndirectOffsetOnAxis(ap=idxi[:, :1], axis=0),
                    bounds_check=K - 1, oob_is_err=False)
                diff = work.tile([P, D], f32, tag="diff")
                nc.vector.tensor_tensor(
                    out=diff[:], in0=q[:], in1=x[:], op=mybir.AluOpType.subtract)
                # st reuses the q tile (q is dead once diff is computed)
                nc.vector.tensor_tensor(
                    out=q[:], in0=x[:], in1=diff[:], op=mybir.AluOpType.add)
                nc.sync.dma_start(out=st_o.ap()[rs, :], in_=q[:])
                sse = small.tile([P, 1], f32, tag="sse")
                nc.scalar.activation(
                    out=diff[:], in_=diff[:],
                    func=mybir.ActivationFunctionType.Square, accum_out=sse[:])
                nc.sync.dma_start(out=sse_o.ap()[:, rt:rt + 1], in_=sse[:])

    nc.compile()
    return nc


_NC_CACHE = None


def _get_nc():
    global _NC_CACHE
    if _NC_CACHE is None:
        _NC_CACHE = build_nc()
    return _NC_CACHE


def run_cores(inputs, weight, trace=False):
    """Shard, run the SPMD kernel on 8 cores, return BassKernelResults."""
    x = np.ascontiguousarray(np.asarray(inputs, dtype=np.float32))
    w = np.ascontiguousarray(np.asarray(weight, dtype=np.float32))
    assert x.shape == (N_INPUTS, D) and w.shape == (K, D)

    wt2 = np.ascontiguousarray((2.0 * w).T)            # exact power-of-2 scale
    wsq = np.sum(np.square(w), axis=1, dtype=np.float32).reshape(1, K)

    in_maps = []
    for c in range(N_CORES):
        xs = x[c * N_CORE:(c + 1) * N_CORE]
        in_maps.append({
            "x": xs,
            "xt": np.ascontiguousarray(xs.T),
            "wt2": wt2,
            "w": w,
            "wsq": wsq,
        })
    nc = _get_nc()
    return run_bass_kernel_spmd(nc, in_maps, core_ids=list(range(N_CORES)),
                                trace=trace)


def assemble(results, inputs):
    """Combine per-core outputs into the reference's return tuple."""
    probs = np.concatenate([r["probs"] for r in results], axis=0)
    quantized_st = np.concatenate([r["st"] for r in results], axis=0)
    idx = np.concatenate([r["idx"][:, 0] for r in results], axis=0)

    sse_total = np.float64(0.0)
    for r in results:
        sse_total += r["sse"].astype(np.float64).sum()
    mse = np.float32(sse_total / (N_INPUTS * D))
    loss = np.float32(mse + np.float32(COMMITMENT_COST) * mse)

    counts = np.bincount(idx, minlength=K).astype(np.float32)
    avg_probs = counts / np.float32(N_INPUTS)
    perplexity = np.float32(
        np.exp(-np.sum(avg_probs * np.log(avg_probs + np.float32(1e-10)))))

    return quantized_st, probs, loss, perplexity


def kernel(**kw):
    inputs = np.asarray(kw["inputs"])
    weight = np.asarray(kw["weight"])
    res = run_cores(inputs, weight, trace=False)
    return assemble(res.results, inputs)


# revision 8
# speedup vs baseline: 1.2777x; 1.2777x over previous
"""VQ codebook (nn_CodeBook) Trainium2 Bass kernel.

Data-parallel over the N=32768 input rows across 8 NeuronCores; the
[8192, 512] codebook is replicated.  Each core computes, for its 4096-row
shard:

  u    = x @ (2W)^T                    (PE, fp32, 4 d-chunks x 16 k-chunks)
  z    = u - ||w||^2                   (DVE, fused with PSUM evacuation)
  m    = rowmax(z)                     (DVE, per-chunk max + combine)
  e    = exp(z - m), s = rowsum(e)     (ACT, fused accumulate)
  probs= e * (1/s)                     (ACT copy with per-partition scale)
  idx  = argmax(z) == argmin(dist)     (DVE, is_ge mask * reverse iota)
  q    = W[idx]                        (indirect DMA gather)
  st   = x + (q - x)                   (straight-through estimator)
  sse  = rowsum((q - x)^2)             (ACT square with accumulate)

Host side: shard/transpose inputs, then assemble probs/quantized_st,
bincount(idx) -> perplexity, and sum(sse) -> loss.  Softmax of
-(||x||^2 + ||w||^2 - 2xw) over k equals softmax of (2xw - ||w||^2)
because the per-row ||x||^2 shifts cancel; the argmin is likewise
unaffected.
"""

import numpy as np

import concourse.bass as bass
import concourse.bacc as bacc
import concourse.tile as tile
import concourse.mybir as mybir
from concourse.bass_utils import run_bass_kernel_spmd

# Problem shape (hardcoded per contract).
N_INPUTS = 32768
K = 8192            # NUM_EMBEDDINGS
D = 512             # EMBEDDING_DIM
COMMITMENT_COST = 0.25

N_CORES = 8
N_CORE = N_INPUTS // N_CORES   # 4096 rows per core
P = 128                        # partitions
NT = N_CORE // P               # 32 row-tiles per core
KC = 512                       # k chunk (one PSUM bank of fp32)
NKC = K // KC                  # 16 chunks
ND = D // P                    # 4 contraction chunks

f32 = mybir.dt.float32
i32 = mybir.dt.int32


def build_nc():
    nc = bacc.Bacc("TRN2", num_devices=N_CORES, dynamic_dma_scratch_size=8192)

    x_d = nc.dram_tensor("x", [N_CORE, D], f32, kind="ExternalInput")
    xt_d = nc.dram_tensor("xt", [D, N_CORE], f32, kind="ExternalInput")
    wt2_d = nc.dram_tensor("wt2", [D, K], f32, kind="ExternalInput")
    w_d = nc.dram_tensor("w", [K, D], f32, kind="ExternalInput")
    wsq_d = nc.dram_tensor("wsq", [1, K], f32, kind="ExternalInput")

    probs_o = nc.dram_tensor("probs", [N_CORE, K], f32, kind="ExternalOutput")
    st_o = nc.dram_tensor("st", [N_CORE, D], f32, kind="ExternalOutput")
    idx_o = nc.dram_tensor("idx", [N_CORE, 1], i32, kind="ExternalOutput")
    sse_o = nc.dram_tensor("sse", [P, NT], f32, kind="ExternalOutput")

    with tile.TileContext(nc) as tc:
        with (
            tc.tile_pool(name="const", bufs=1) as const,
            tc.tile_pool(name="zc", bufs=16) as zpool,
            tc.tile_pool(name="xin", bufs=2) as xin,
            tc.tile_pool(name="work", bufs=2) as work,
            tc.tile_pool(name="small", bufs=2) as small,
            tc.tile_pool(name="psum", bufs=6, space="PSUM") as psum,
            tc.tile_pool(name="ptmp", bufs=2, space="PSUM") as ptmp,
        ):
            # ---------------- resident constants ----------------
            wt2 = []
            for d in range(ND):
                t = const.tile([P, K], f32, tag=f"wt2_{d}")
                nc.sync.dma_start(out=t[:], in_=wt2_d.ap()[d * P:(d + 1) * P, :])
                wt2.append(t)
            wsqb = const.tile([P, K], f32)
            nc.sync.dma_start(
                out=wsqb[:],
                in_=bass.AP(tensor=wsq_d.ap().tensor, offset=0, ap=[[0, P], [1, K]]),
            )
            iota_rev = const.tile([P, KC], i32)   # KC..1
            nc.gpsimd.iota(iota_rev[:], [[-1, KC]], base=KC, channel_multiplier=0)
            iota_asc_c = const.tile([P, NKC], i32)  # 1..NKC
            nc.gpsimd.iota(iota_asc_c[:], [[1, NKC]], base=1, channel_multiplier=0)
            iota_off_c = const.tile([P, NKC], i32)  # 1024*(1..NKC)
            nc.gpsimd.iota(iota_off_c[:], [[1024, NKC]], base=1024,
                           channel_multiplier=0)

            for rt in range(NT):
                rs = slice(rt * P, (rt + 1) * P)
                # ---------------- loads ----------------
                xt = []
                for d in range(ND):
                    t = xin.tile([P, P], f32, tag=f"xt_{d}")
                    nc.sync.dma_start(
                        out=t[:],
                        in_=xt_d.ap()[d * P:(d + 1) * P, rs])
                    xt.append(t)
                x = xin.tile([P, D], f32, tag="x")
                nc.sync.dma_start(out=x[:], in_=x_d.ap()[rs, :])

                # ------- matmul + evac + online prefix-max exp + argmax masks ----
                # For each chunk: z_c = u_c - wsq_c; Mpre_c = running rowmax;
                # e_c = exp(z_c - Mpre_c) with chunk sum S_c.  The global-max
                # element maps to exp(0) == 1.0 exactly in the LAST chunk that
                # improves the prefix max; masks (e >= 1) * rev-iota find it.
                zs = []
                Mpre = small.tile([P, NKC], f32, tag="Mpre")
                negMpre = small.tile([P, NKC], f32, tag="negMpre")
                S = small.tile([P, NKC], f32, tag="S")
                J = small.tile([P, NKC], f32, tag="J")
                for c in range(NKC):
                    cs = slice(c * KC, (c + 1) * KC)
                    pu = psum.tile([P, KC], f32, tag="pu")
                    for d in range(ND):
                        nc.tensor.matmul(
                            pu[:], lhsT=xt[d][:], rhs=wt2[d][:, cs],
                            start=(d == 0), stop=(d == ND - 1))
                    z = zpool.tile([P, KC], f32, tag="z")
                    zs.append(z)
                    # z_c = pu - wsq_c  (PSUM -> SBUF)
                    nc.vector.scalar_tensor_tensor(
                        out=z[:], in0=pu[:], scalar=1.0, in1=wsqb[:, cs],
                        op0=mybir.AluOpType.mult, op1=mybir.AluOpType.subtract)
                    nc.vector.tensor_reduce(
                        out=Mpre[:, c:c + 1], in_=z[:], axis=mybir.AxisListType.X,
                        op=mybir.AluOpType.max)
                    if c > 0:
                        nc.vector.tensor_tensor(
                            out=Mpre[:, c:c + 1], in0=Mpre[:, c:c + 1],
                            in1=Mpre[:, c - 1:c], op=mybir.AluOpType.max)
                    nc.vector.tensor_scalar_mul(
                        negMpre[:, c:c + 1], Mpre[:, c:c + 1], -1.0)
                    # e_c = exp(z_c - Mpre_c) in place, chunk sum into S_c
                    nc.scalar.activation(
                        out=z[:], in_=z[:], func=mybir.ActivationFunctionType.Exp,
                        bias=negMpre[:, c:c + 1], scale=1.0,
                        accum_out=S[:, c:c + 1])
                    # prefix-max hits: (e_c >= 1) * (KC..1) -> J_c
                    tmp = ptmp.tile([P, KC], f32, tag="tmp", space="PSUM")
                    nc.vector.scalar_tensor_tensor(
                        out=tmp[:], in0=z[:], scalar=1.0, in1=iota_rev[:],
                        op0=mybir.AluOpType.is_ge, op1=mybir.AluOpType.mult)
                    nc.vector.tensor_reduce(
                        out=J[:, c:c + 1], in_=tmp[:], axis=mybir.AxisListType.X,
                        op=mybir.AluOpType.max)

                # ------- combine: s = sum_c S_c * exp(Mpre_c - m); alpha ------
                m = Mpre[:, NKC - 1:NKC]
                beta = small.tile([P, NKC], f32, tag="beta")
                negm = small.tile([P, 1], f32, tag="negm")
                nc.vector.tensor_scalar_mul(negm[:], m, -1.0)
                nc.scalar.activation(
                    out=beta[:], in_=Mpre[:],
                    func=mybir.ActivationFunctionType.Exp,
                    bias=negm[:], scale=1.0)
                Sb = small.tile([P, NKC], f32, tag="Sb")
                nc.vector.tensor_tensor(
                    out=Sb[:], in0=S[:], in1=beta[:], op=mybir.AluOpType.mult)
                s = small.tile([P, 1], f32, tag="s")
                nc.vector.tensor_reduce(
                    out=s[:], in_=Sb[:], axis=mybir.AxisListType.X,
                    op=mybir.AluOpType.add)
                r = small.tile([P, 1], f32, tag="r")
                nc.vector.reciprocal(r[:], s[:])
                alpha = small.tile([P, NKC], f32, tag="alpha")
                nc.vector.tensor_scalar_mul(alpha[:], beta[:], r[:])
                for c in range(NKC):
                    cs = slice(c * KC, (c + 1) * KC)
                    z = zs[c]
                    nc.scalar.activation(
                        out=z[:], in_=z[:], func=mybir.ActivationFunctionType.Copy,
                        bias=0.0, scale=alpha[:, c:c + 1])
                    nc.sync.dma_start(out=probs_o.ap()[rs, cs], in_=z[:])

                # ------- index: g = max_c((c+1)*1024 + J_c | J_c>0) ----------
                # C = last improving chunk (1-based); idx = 1536*C - g
                T1 = small.tile([P, NKC], f32, tag="T1")
                nc.vector.tensor_tensor(
                    out=T1[:], in0=J[:], in1=iota_off_c[:], op=mybir.AluOpType.add)
                G = small.tile([P, NKC], f32, tag="G")
                nc.vector.scalar_tensor_tensor(
                    out=G[:], in0=J[:], scalar=0.0, in1=T1[:],
                    op0=mybir.AluOpType.is_gt, op1=mybir.AluOpType.mult)
                g = small.tile([P, 1], f32, tag="g")
                nc.vector.tensor_reduce(
                    out=g[:], in_=G[:], axis=mybir.AxisListType.X,
                    op=mybir.AluOpType.max)
                Ca = small.tile([P, NKC], f32, tag="Ca")
                nc.vector.scalar_tensor_tensor(
                    out=Ca[:], in0=J[:], scalar=0.0, in1=iota_asc_c[:],
                    op0=mybir.AluOpType.is_gt, op1=mybir.AluOpType.mult)
                Cm = small.tile([P, 1], f32, tag="Cm")
                nc.vector.tensor_reduce(
                    out=Cm[:], in_=Ca[:], axis=mybir.AxisListType.X,
                    op=mybir.AluOpType.max)
                idxf = small.tile([P, 1], f32, tag="idxf")
                nc.vector.scalar_tensor_tensor(
                    out=idxf[:], in0=Cm[:], scalar=1536.0, in1=g[:],
                    op0=mybir.AluOpType.mult, op1=mybir.AluOpType.subtract)
                idxi = small.tile([P, 1], i32, tag="idxi")
                nc.vector.tensor_copy(out=idxi[:], in_=idxf[:])
                nc.sync.dma_start(out=idx_o.ap()[rs, :], in_=idxi[:])

                # ---------------- gather / straight-through / sse ----------------
                q = work.tile([P, D], f32, tag="q")
                nc.gpsimd.indirect_dma_start(
                    out=q[:], out_offset=None, in_=w_d.ap(),
                    in_offset=bass.IndirectOffsetOnAxis(ap=idxi[:, :1], axis=0),
                    bounds_check=K - 1, oob_is_err=False)
                diff = work.tile([P, D], f32, tag="diff")
                nc.vector.tensor_tensor(
                    out=diff[:], in0=q[:], in1=x[:], op=mybir.AluOpType.subtract)
                # st reuses the q tile (q is dead once diff is computed)
                nc.vector.tensor_tensor(
                    out=q[:], in0=x[:], in1=diff[:], op=mybir.AluOpType.add)
                nc.sync.dma_start(out=st_o.ap()[rs, :], in_=q[:])
                sse = small.tile([P, 1], f32, tag="sse")
                nc.scalar.activation(
                    out=diff[:], in_=diff[:],
                    func=mybir.ActivationFunctionType.Square, accum_out=sse[:])
                nc.sync.dma_start(out=sse_o.ap()[:, rt:rt + 1], in_=sse[:])

    nc.compile()
    return nc


_NC_CACHE = None


def _get_nc():
    global _NC_CACHE
    if _NC_CACHE is None:
        _NC_CACHE = build_nc()
    return _NC_CACHE


def run_cores(inputs, weight, trace=False):
    """Shard, run the SPMD kernel on 8 cores, return BassKernelResults."""
    x = np.ascontiguousarray(np.asarray(inputs, dtype=np.float32))
    w = np.ascontiguousarray(np.asarray(weight, dtype=np.float32))
    assert x.shape == (N_INPUTS, D) and w.shape == (K, D)

    wt2 = np.ascontiguousarray((2.0 * w).T)            # exact power-of-2 scale
    wsq = np.sum(np.square(w), axis=1, dtype=np.float32).reshape(1, K)

    in_maps = []
    for c in range(N_CORES):
        xs = x[c * N_CORE:(c + 1) * N_CORE]
        in_maps.append({
            "x": xs,
            "xt": np.ascontiguousarray(xs.T),
            "wt2": wt2,
            "w": w,
            "wsq": wsq,
        })
    nc = _get_nc()
    return run_bass_kernel_spmd(nc, in_maps, core_ids=list(range(N_CORES)),
                                trace=trace)


def assemble(results, inputs):
    """Combine per-core outputs into the reference's return tuple."""
    probs = np.concatenate([r["probs"] for r in results], axis=0)
    quantized_st = np.concatenate([r["st"] for r in results], axis=0)
    idx = np.concatenate([r["idx"][:, 0] for r in results], axis=0)

    sse_total = np.float64(0.0)
    for r in results:
        sse_total += r["sse"].astype(np.float64).sum()
    mse = np.float32(sse_total / (N_INPUTS * D))
    loss = np.float32(mse + np.float32(COMMITMENT_COST) * mse)

    counts = np.bincount(idx, minlength=K).astype(np.float32)
    avg_probs = counts / np.float32(N_INPUTS)
    perplexity = np.float32(
        np.exp(-np.sum(avg_probs * np.log(avg_probs + np.float32(1e-10)))))

    return quantized_st, probs, loss, perplexity


def kernel(**kw):
    inputs = np.asarray(kw["inputs"])
    weight = np.asarray(kw["weight"])
    res = run_cores(inputs, weight, trace=False)
    return assemble(res.results, inputs)


# revision 10
# speedup vs baseline: 1.3082x; 1.0239x over previous
"""VQ codebook (nn_CodeBook) Trainium2 Bass kernel.

Data-parallel over the N=32768 input rows across 8 NeuronCores; the
[8192, 512] codebook is replicated.  Each core computes, for its 4096-row
shard:

  u    = x @ (2W)^T                    (PE, fp32, 4 d-chunks x 16 k-chunks)
  z    = u - ||w||^2                   (DVE, fused with PSUM evacuation)
  m    = rowmax(z)                     (DVE, per-chunk max + combine)
  e    = exp(z - m), s = rowsum(e)     (ACT, fused accumulate)
  probs= e * (1/s)                     (ACT copy with per-partition scale)
  idx  = argmax(z) == argmin(dist)     (DVE, is_ge mask * reverse iota)
  q    = W[idx]                        (indirect DMA gather)
  st   = x + (q - x)                   (straight-through estimator)
  sse  = rowsum((q - x)^2)             (ACT square with accumulate)

Host side: shard/transpose inputs, then assemble probs/quantized_st,
bincount(idx) -> perplexity, and sum(sse) -> loss.  Softmax of
-(||x||^2 + ||w||^2 - 2xw) over k equals softmax of (2xw - ||w||^2)
because the per-row ||x||^2 shifts cancel; the argmin is likewise
unaffected.
"""

import numpy as np

import concourse.bass as bass
import concourse.bacc as bacc
import concourse.tile as tile
import concourse.mybir as mybir
from concourse.bass_utils import run_bass_kernel_spmd

# Problem shape (hardcoded per contract).
N_INPUTS = 32768
K = 8192            # NUM_EMBEDDINGS
D = 512             # EMBEDDING_DIM
COMMITMENT_COST = 0.25

N_CORES = 8
N_CORE = N_INPUTS // N_CORES   # 4096 rows per core
P = 128                        # partitions
NT = N_CORE // P               # 32 row-tiles per core
KC = 512                       # k chunk (one PSUM bank of fp32)
NKC = K // KC                  # 16 chunks
ND = D // P                    # 4 contraction chunks

f32 = mybir.dt.float32
i32 = mybir.dt.int32


def build_nc():
    nc = bacc.Bacc("TRN2", num_devices=N_CORES, dynamic_dma_scratch_size=8192)

    x_d = nc.dram_tensor("x", [N_CORE, D], f32, kind="ExternalInput")
    xt_d = nc.dram_tensor("xt", [D, N_CORE], f32, kind="ExternalInput")
    wt2_d = nc.dram_tensor("wt2", [D, K], f32, kind="ExternalInput")
    w_d = nc.dram_tensor("w", [K, D], f32, kind="ExternalInput")
    wsq_d = nc.dram_tensor("wsq", [1, K], f32, kind="ExternalInput")

    probs_o = nc.dram_tensor("probs", [N_CORE, K], f32, kind="ExternalOutput")
    st_o = nc.dram_tensor("st", [N_CORE, D], f32, kind="ExternalOutput")
    idx_o = nc.dram_tensor("idx", [N_CORE, 1], i32, kind="ExternalOutput")
    sse_o = nc.dram_tensor("sse", [P, NT], f32, kind="ExternalOutput")

    with tile.TileContext(nc) as tc:
        with (
            tc.tile_pool(name="const", bufs=1) as const,
            tc.tile_pool(name="zc", bufs=16) as zpool,
            tc.tile_pool(name="xin", bufs=2) as xin,
            tc.tile_pool(name="work", bufs=2) as work,
            tc.tile_pool(name="small", bufs=2) as small,
            tc.tile_pool(name="psum", bufs=6, space="PSUM") as psum,
            tc.tile_pool(name="ptmp", bufs=2, space="PSUM") as ptmp,
        ):
            # ---------------- resident constants ----------------
            # chunked [P, KC] tiles so the first matmuls only wait for the
            # first chunk-column of each d instead of the full 16 MiB
            wt2 = [[None] * NKC for _ in range(ND)]
            wsqb = [None] * NKC
            for c in range(NKC):
                for d in range(ND):
                    t = const.tile([P, KC], f32, tag=f"wt2_{d}_{c}",
                                   name=f"wt2_{d}_{c}")
                    nc.sync.dma_start(
                        out=t[:],
                        in_=wt2_d.ap()[d * P:(d + 1) * P, c * KC:(c + 1) * KC])
                    wt2[d][c] = t
                t = const.tile([P, KC], f32, tag=f"wsqb_{c}", name=f"wsqb_{c}")
                nc.sync.dma_start(
                    out=t[:],
                    in_=bass.AP(tensor=wsq_d.ap().tensor, offset=c * KC,
                                ap=[[0, P], [1, KC]]))
                wsqb[c] = t
            iota_rev = const.tile([P, KC], i32)   # KC..1
            nc.gpsimd.iota(iota_rev[:], [[-1, KC]], base=KC, channel_multiplier=0)
            iota_asc_c = const.tile([P, NKC], i32)  # 1..NKC
            nc.gpsimd.iota(iota_asc_c[:], [[1, NKC]], base=1, channel_multiplier=0)
            iota_off_c = const.tile([P, NKC], i32)  # 1024*(1..NKC)
            nc.gpsimd.iota(iota_off_c[:], [[1024, NKC]], base=1024,
                           channel_multiplier=0)

            for rt in range(NT):
                rs = slice(rt * P, (rt + 1) * P)
                # ---------------- loads ----------------
                xt = []
                for d in range(ND):
                    t = xin.tile([P, P], f32, tag=f"xt_{d}")
                    nc.sync.dma_start(
                        out=t[:],
                        in_=xt_d.ap()[d * P:(d + 1) * P, rs])
                    xt.append(t)
                x = xin.tile([P, D], f32, tag="x")
                nc.sync.dma_start(out=x[:], in_=x_d.ap()[rs, :])

                # ------- matmul + evac + online prefix-max exp + argmax masks ----
                # For each chunk: z_c = u_c - wsq_c; Mpre_c = running rowmax;
                # e_c = exp(z_c - Mpre_c) with chunk sum S_c.  The global-max
                # element maps to exp(0) == 1.0 exactly in the LAST chunk that
                # improves the prefix max; masks (e >= 1) * rev-iota find it.
                zs = []
                Mpre = small.tile([P, NKC], f32, tag="Mpre")
                negMpre = small.tile([P, NKC], f32, tag="negMpre")
                S = small.tile([P, NKC], f32, tag="S")
                J = small.tile([P, NKC], f32, tag="J")
                for c in range(NKC):
                    pu = psum.tile([P, KC], f32, tag="pu")
                    for d in range(ND):
                        nc.tensor.matmul(
                            pu[:], lhsT=xt[d][:], rhs=wt2[d][c][:],
                            start=(d == 0), stop=(d == ND - 1))
                    z = zpool.tile([P, KC], f32, tag="z")
                    zs.append(z)
                    # z_c = pu - wsq_c  (PSUM -> SBUF)
                    nc.vector.scalar_tensor_tensor(
                        out=z[:], in0=pu[:], scalar=1.0, in1=wsqb[c][:],
                        op0=mybir.AluOpType.mult, op1=mybir.AluOpType.subtract)
                    nc.vector.tensor_reduce(
                        out=Mpre[:, c:c + 1], in_=z[:], axis=mybir.AxisListType.X,
                        op=mybir.AluOpType.max)
                    if c > 0:
                        nc.vector.tensor_tensor(
                            out=Mpre[:, c:c + 1], in0=Mpre[:, c:c + 1],
                            in1=Mpre[:, c - 1:c], op=mybir.AluOpType.max)
                    nc.vector.tensor_scalar_mul(
                        negMpre[:, c:c + 1], Mpre[:, c:c + 1], -1.0)
                    # e_c = exp(z_c - Mpre_c) in place, chunk sum into S_c
                    nc.scalar.activation(
                        out=z[:], in_=z[:], func=mybir.ActivationFunctionType.Exp,
                        bias=negMpre[:, c:c + 1], scale=1.0,
                        accum_out=S[:, c:c + 1])
                # prefix-max hits in a second loop so PSUM evacuations keep
                # scheduler priority over mask work
                for c in range(NKC):
                    z = zs[c]
                    tmp = ptmp.tile([P, KC], f32, tag="tmp", space="PSUM")
                    nc.vector.scalar_tensor_tensor(
                        out=tmp[:], in0=z[:], scalar=1.0, in1=iota_rev[:],
                        op0=mybir.AluOpType.is_ge, op1=mybir.AluOpType.mult)
                    nc.vector.tensor_reduce(
                        out=J[:, c:c + 1], in_=tmp[:], axis=mybir.AxisListType.X,
                        op=mybir.AluOpType.max)

                # ------- combine: s = sum_c S_c * exp(Mpre_c - m); alpha ------
                m = Mpre[:, NKC - 1:NKC]
                beta = small.tile([P, NKC], f32, tag="beta")
                negm = small.tile([P, 1], f32, tag="negm")
                nc.vector.tensor_scalar_mul(negm[:], m, -1.0)
                nc.scalar.activation(
                    out=beta[:], in_=Mpre[:],
                    func=mybir.ActivationFunctionType.Exp,
                    bias=negm[:], scale=1.0)
                Sb = small.tile([P, NKC], f32, tag="Sb")
                nc.vector.tensor_tensor(
                    out=Sb[:], in0=S[:], in1=beta[:], op=mybir.AluOpType.mult)
                s = small.tile([P, 1], f32, tag="s")
                nc.vector.tensor_reduce(
                    out=s[:], in_=Sb[:], axis=mybir.AxisListType.X,
                    op=mybir.AluOpType.add)
                r = small.tile([P, 1], f32, tag="r")
                nc.vector.reciprocal(r[:], s[:])
                alpha = small.tile([P, NKC], f32, tag="alpha")
                nc.vector.tensor_scalar_mul(alpha[:], beta[:], r[:])
                for c in range(NKC):
                    cs = slice(c * KC, (c + 1) * KC)
                    z = zs[c]
                    nc.scalar.activation(
                        out=z[:], in_=z[:], func=mybir.ActivationFunctionType.Copy,
                        bias=0.0, scale=alpha[:, c:c + 1])
                    nc.sync.dma_start(out=probs_o.ap()[rs, cs], in_=z[:])

                # ------- index: g = max_c((c+1)*1024 + J_c | J_c>0) ----------
                # C = last improving chunk (1-based); idx = 1536*C - g
                T1 = small.tile([P, NKC], f32, tag="T1")
                nc.vector.tensor_tensor(
                    out=T1[:], in0=J[:], in1=iota_off_c[:], op=mybir.AluOpType.add)
                G = small.tile([P, NKC], f32, tag="G")
                nc.vector.scalar_tensor_tensor(
                    out=G[:], in0=J[:], scalar=0.0, in1=T1[:],
                    op0=mybir.AluOpType.is_gt, op1=mybir.AluOpType.mult)
                g = small.tile([P, 1], f32, tag="g")
                nc.vector.tensor_reduce(
                    out=g[:], in_=G[:], axis=mybir.AxisListType.X,
                    op=mybir.AluOpType.max)
                Ca = small.tile([P, NKC], f32, tag="Ca")
                nc.vector.scalar_tensor_tensor(
                    out=Ca[:], in0=J[:], scalar=0.0, in1=iota_asc_c[:],
                    op0=mybir.AluOpType.is_gt, op1=mybir.AluOpType.mult)
                Cm = small.tile([P, 1], f32, tag="Cm")
                nc.vector.tensor_reduce(
                    out=Cm[:], in_=Ca[:], axis=mybir.AxisListType.X,
                    op=mybir.AluOpType.max)
                idxf = small.tile([P, 1], f32, tag="idxf")
                nc.vector.scalar_tensor_tensor(
                    out=idxf[:], in0=Cm[:], scalar=1536.0, in1=g[:],
                    op0=mybir.AluOpType.mult, op1=mybir.AluOpType.subtract)
                idxi = small.tile([P, 1], i32, tag="idxi")
                nc.vector.tensor_copy(out=idxi[:], in_=idxf[:])
                nc.sync.dma_start(out=idx_o.ap()[rs, :], in_=idxi[:])

                # ---------------- gather / straight-through / sse ----------------
                q = work.tile([P, D], f32, tag="q")
                nc.gpsimd.indirect_dma_start(
                    out=q[:], out_offset=None, in_=w_d.ap(),
                    in_offset=bass.IndirectOffsetOnAxis(ap=idxi[:, :1], axis=0),
                    bounds_check=K - 1, oob_is_err=False)
                diff = work.tile([P, D], f32, tag="diff")
                nc.vector.tensor_tensor(
                    out=diff[:], in0=q[:], in1=x[:], op=mybir.AluOpType.subtract)
                # st reuses the q tile (q is dead once diff is computed)
                nc.vector.tensor_tensor(
                    out=q[:], in0=x[:], in1=diff[:], op=mybir.AluOpType.add)
                nc.sync.dma_start(out=st_o.ap()[rs, :], in_=q[:])
                sse = small.tile([P, 1], f32, tag="sse")
                nc.scalar.activation(
                    out=diff[:], in_=diff[:],
                    func=mybir.ActivationFunctionType.Square, accum_out=sse[:])
                nc.sync.dma_start(out=sse_o.ap()[:, rt:rt + 1], in_=sse[:])

    nc.compile()
    return nc


_NC_CACHE = None


def _get_nc():
    global _NC_CACHE
    if _NC_CACHE is None:
        _NC_CACHE = build_nc()
    return _NC_CACHE


def run_cores(inputs, weight, trace=False):
    """Shard, run the SPMD kernel on 8 cores, return BassKernelResults."""
    x = np.ascontiguousarray(np.asarray(inputs, dtype=np.float32))
    w = np.ascontiguousarray(np.asarray(weight, dtype=np.float32))
    assert x.shape == (N_INPUTS, D) and w.shape == (K, D)

    wt2 = np.ascontiguousarray((2.0 * w).T)            # exact power-of-2 scale
    wsq = np.sum(np.square(w), axis=1, dtype=np.float32).reshape(1, K)

    in_maps = []
    for c in range(N_CORES):
        xs = x[c * N_CORE:(c + 1) * N_CORE]
        in_maps.append({
            "x": xs,
            "xt": np.ascontiguousarray(xs.T),
            "wt2": wt2,
            "w": w,
            "wsq": wsq,
        })
    nc = _get_nc()
    return run_bass_kernel_spmd(nc, in_maps, core_ids=list(range(N_CORES)),
                                trace=trace)


def assemble(results, inputs):
    """Combine per-core outputs into the reference's return tuple."""
    probs = np.concatenate([r["probs"] for r in results], axis=0)
    quantized_st = np.concatenate([r["st"] for r in results], axis=0)
    idx = np.concatenate([r["idx"][:, 0] for r in results], axis=0)

    sse_total = np.float64(0.0)
    for r in results:
        sse_total += r["sse"].astype(np.float64).sum()
    mse = np.float32(sse_total / (N_INPUTS * D))
    loss = np.float32(mse + np.float32(COMMITMENT_COST) * mse)

    counts = np.bincount(idx, minlength=K).astype(np.float32)
    avg_probs = counts / np.float32(N_INPUTS)
    perplexity = np.float32(
        np.exp(-np.sum(avg_probs * np.log(avg_probs + np.float32(1e-10)))))

    return quantized_st, probs, loss, perplexity


def kernel(**kw):
    inputs = np.asarray(kw["inputs"])
    weight = np.asarray(kw["weight"])
    res = run_cores(inputs, weight, trace=False)
    return assemble(res.results, inputs)
